# revision 25
# baseline (speedup 1.0000x reference)
"""Bahdanau additive attention on 8 Trainium2 NeuronCores.

  score_t = V^T tanh(W1 value_t + W2 query);  out = softmax(score) @ value

Sharding: data-parallel over batch (16 batches -> 2 per core), weights
replicated.

v5: the xbar DMA-transposes are serialized against ALL other DMA traffic
(compiler deadlock guard), and lone loads interleaved between transposes
drop to ~200 GB/s, so the DMA schedule must be phased: all value loads
back-to-back at full rate first, then the xbar transposes.  To shrink
both the idle-PE load phase and the xbar chain:
  * supertiles j<3 (both batches) are transposed ON THE PE ARRAY during
    the load phase (16x 128x128 transpose-mode matmuls + one PSUM->SBUF
    copy per d-half, split DVE/ACT), so their keys/tanh/scores complete
    before the first xbar; only j>=3 (10 supertiles) use the xbar.
  * softmax + context accumulation run incrementally in four j-pair
    groups: score-scatter SBUF->SBUF DMAs execute between xbars on the
    SP ring (dep-sandwiched -- never concurrent with a transpose), so
    the context matmuls overlap the xbar phase instead of serializing
    at the end.
"""

import functools
import os
import sys

import numpy as np

if "/opt/trn_rl_repo" not in sys.path:
    sys.path.insert(0, "/opt/trn_rl_repo")

B, T, D, U = 16, 8192, 256, 256
NCORES = 8
BPC = B // NCORES          # batches per core
P = 128                    # SBUF partitions
ST = 1024                  # t per supertile
NST = T // ST              # supertiles per batch
CH = 512                   # score/keys chunk width (PSUM bank = 512 fp32)
NCH = ST // CH             # chunks per supertile
NPE = 4                    # j < NPE transpose on the PE array (per batch)
NG = 4                     # softmax/context groups (j-pairs)


@functools.lru_cache(maxsize=1)
def _build():
    from contextlib import ExitStack

    import concourse.bass as bass
    import concourse.tile as tile
    from concourse import bacc, mybir
    from concourse.masks import make_identity

    f32 = mybir.dt.float32
    bf16 = mybir.dt.bfloat16
    Act = mybir.ActivationFunctionType

    nc = bacc.Bacc("TRN2", target_bir_lowering=False, debug=False)

    q = nc.dram_tensor("query", [BPC, D], f32, kind="ExternalInput").ap()
    val = nc.dram_tensor("value", [BPC, T, D], f32, kind="ExternalInput").ap()
    w1 = nc.dram_tensor("W1", [D, U], f32, kind="ExternalInput").ap()
    w2 = nc.dram_tensor("W2", [D, U], f32, kind="ExternalInput").ap()
    vv = nc.dram_tensor("V", [U, 1], f32, kind="ExternalInput").ap()
    out = nc.dram_tensor("out", [BPC, D], f32, kind="ExternalOutput").ap()

    with tile.TileContext(nc) as tc, ExitStack() as ctx:
        consts = ctx.enter_context(tc.tile_pool(name="consts", bufs=1))
        vpool = ctx.enter_context(tc.tile_pool(name="vbf", bufs=1))
        v32pool = ctx.enter_context(tc.tile_pool(name="v32", bufs=4))
        vtpool = ctx.enter_context(tc.tile_pool(name="vt", bufs=3))
        thpool = ctx.enter_context(tc.tile_pool(name="th", bufs=3))
        scpool = ctx.enter_context(tc.tile_pool(name="scsb", bufs=1))
        small = ctx.enter_context(tc.tile_pool(name="small", bufs=1))
        psk = ctx.enter_context(tc.tile_pool(name="psk", bufs=2, space="PSUM"))
        pssc = ctx.enter_context(tc.tile_pool(name="pssc", bufs=1, space="PSUM"))
        pst = ctx.enter_context(tc.tile_pool(name="pst", bufs=1, space="PSUM"))
        psctx = ctx.enter_context(tc.tile_pool(name="psctx", bufs=1, space="PSUM"))
        psmisc = ctx.enter_context(tc.tile_pool(name="psmisc", bufs=1, space="PSUM"))

        # ---- constants / weights -------------------------------------
        ident = consts.tile([64, 64], f32)
        make_identity(nc, ident)
        identb = consts.tile([P, P], bf16)
        make_identity(nc, identb)
        ones = consts.tile([P, 1], f32)
        nc.gpsimd.memset(ones, 1.0)
        # preload the exp/tanh ACT table set (~2.7us) during the first
        # value loads instead of stalling the first real tanh on it.
        warm = consts.tile([P, 1], f32)
        nc.scalar.activation(out=warm, in_=warm, func=Act.Tanh, scale=1.0)

        # Small weight loads go FIRST on the SP ring, ahead of the value
        # loads: any DMA issued alongside the saturated value-load stream
        # completes ~25us late and would gate the first PE ops.
        w1tmp = consts.tile([P, 2, U], f32)
        nc.sync.dma_start(out=w1tmp, in_=w1.rearrange("(kb p) u -> p kb u", p=P))
        w1b = consts.tile([P, 2, U], bf16)
        nc.vector.tensor_copy(out=w1b, in_=w1tmp)
        w2b = consts.tile([P, 2, U], f32)
        nc.sync.dma_start(out=w2b, in_=w2.rearrange("(kb p) u -> p kb u", p=P))
        vtmp = consts.tile([P, 2, 1], f32)
        nc.sync.dma_start(out=vtmp, in_=vv.rearrange("(ub p) o -> p ub o", p=P))
        vsb = consts.tile([P, 2, 1], bf16)
        nc.vector.tensor_copy(out=vsb, in_=vtmp)

        # hidden = query @ W2, computed as hidden^T [u, b] so it can feed
        # the tanh as a per-partition bias.
        q_sb = consts.tile([BPC, D], f32)
        nc.sync.dma_start(out=q_sb, in_=q)
        qt = consts.tile([P, 2, BPC], f32)
        for kb in range(2):
            psq = psmisc.tile([P, BPC], f32, tag="misc", name="psq")
            nc.tensor.transpose(
                out=psq, in_=q_sb[:, P * kb:P * (kb + 1)], identity=ident[0:BPC, 0:BPC]
            )
            nc.scalar.copy(out=qt[:, kb, :], in_=psq)
        hid = []
        for u in range(2):
            psh = psmisc.tile([P, BPC], f32, tag="misc", name="psh")
            for kb in range(2):
                nc.tensor.matmul(
                    psh,
                    lhsT=w2b[:, kb, P * u:P * (u + 1)],
                    rhs=qt[:, kb, :],
                    start=(kb == 0),
                    stop=(kb == 1),
                )
            h = consts.tile([P, BPC], f32, tag=f"hid{u}")
            nc.scalar.copy(out=h, in_=psh)
            hid.append(h)

        ctx_sb = consts.tile([P, D], f32)

        # ---- per-core state ------------------------------------------
        # sc_sb[b][p, c, j, kk, 128]: scores, laid out so one scatter
        # source (c, a j-pair, all kk, f) is a single contiguous run.
        # Only rows 64b (chunk c=0) and 64b+32 (c=1) carry data.
        sc_sb = [
            scpool.tile([P, NCH, NST, 4, P], f32, tag=f"scsb{b}", name=f"scsb{b}")
            for b in range(BPC)
        ]
        # s32[b][g]: scatter target for j-pair group g (j = 2g + jj),
        # row 8*c + 4*jj + k <-> score chunk (t-offset 512c + 128k).
        s32 = [
            [
                small.tile([16, P], f32, tag=f"s32_{b}_{g}", name=f"s32_{b}_{g}")
                for g in range(NG)
            ]
            for b in range(BPC)
        ]
        e128 = [
            small.tile([P, 64], bf16, tag=f"e_{b}", name=f"e_{b}")
            for b in range(BPC)
        ]
        vb = [[None] * NST for _ in range(BPC)]
        vb32s = {}
        # context accumulator: batch b at [32b : 32b+1, 256b : 256b+256]
        # (one PSUM bank, different col groups so the M=1 chains overlap).
        psC = psctx.tile([P, 2 * D], f32, tag="psC", name="psC")

        last_xbar = [None]

        # ---- phase 1: issue ALL value loads (SP ring, back-to-back) --
        # The xbar transposes serialize against every outstanding DMA, so
        # the loads must form one uninterrupted full-bandwidth stream
        # before the first transpose.
        for j in range(NST):
            for b in range(BPC):
                VB32 = v32pool.tile([P, ST // P, D], f32, tag="vb32", name="vb32")
                vb32s[(b, j)] = VB32
                nc.sync.dma_start(
                    out=VB32,
                    in_=val[b, ST * j:ST * (j + 1), :].rearrange(
                        "(p f) d -> p f d", f=ST // P
                    ),
                )

        def stream_supertile(b, j):
            VB32 = vb32s[(b, j)]
            # bf16 cast; each d-half becomes a contiguous 128-run:
            # VB[p, h, f, d'] = value[ST*j + 8p + f, 128h + d']
            VB = vpool.tile([P, 2, ST // P, P], bf16, tag=f"vb_{b}_{j}")
            vb[b][j] = VB
            # PE-path casts on gpsimd (otherwise idle): keeps the DVE
            # completion counter free of cast entanglement so the value
            # loads (which wait on cast slot recycling) stream at full
            # rate.  Xbar-path casts stay on the (then-idle) DVE.
            cast_eng = nc.gpsimd if j < NPE else nc.vector
            cast_eng.tensor_copy(
                out=VB,
                in_=VB32.rearrange("p f (h d) -> p h f d", h=2),
            )
            # VT[d', (h f), t'] = VB[t', h, f, d']
            VT = vtpool.tile([P, 2, ST // P, P], bf16, tag="vt", name="vt")
            if j < NPE:
                # PE transpose-mode path (runs during the load phase,
                # before any xbar): 8 128x128 transposes per d-half into
                # one PSUM bank, then one PSUM->SBUF copy (DVE for h=0,
                # ACT for h=1 so neither engine is the bottleneck).
                for h in range(2):
                    psT = pst.tile([P, ST], bf16, tag="psT", name="psT")
                    for f in range(ST // P):
                        nc.tensor.transpose(
                            out=psT[:, P * f:P * (f + 1)],
                            in_=VB[:, h, f, :],
                            identity=identb,
                        )
                    if h == 0:
                        nc.vector.tensor_copy(out=VT[:, h, :, :], in_=psT)
                    else:
                        nc.scalar.copy(out=VT[:, h, :, :], in_=psT)
            else:
                tr = nc.sync.dma_start(
                    out=VT.rearrange("p h f t -> p (h f) t"),
                    in_=VB.rearrange("p h f t -> p (h f t)"),
                    transpose=True,
                )
                last_xbar[0] = tr
            # keys^T = W1^T @ value^T, one u-half at a time; tanh with the
            # query bias fused reads each 2-bank psK tile in one op.
            ths = []
            for u in range(2):
                psK = psk.tile([P, ST], f32, tag="psK", name="psk")
                for c in range(NCH):
                    for kb in range(2):
                        nc.tensor.matmul(
                            psK[:, CH * c:CH * (c + 1)],
                            lhsT=w1b[:, kb, P * u:P * (u + 1)],
                            rhs=VT[
                                :, kb, (CH // P) * c:(CH // P) * (c + 1), :
                            ].rearrange("p f t -> p (f t)"),
                            start=(kb == 0),
                            stop=(kb == 1),
                        )
                th = thpool.tile([P, ST], bf16, tag="th", name="th")
                nc.scalar.activation(
                    out=th,
                    in_=psK,
                    func=Act.Tanh,
                    bias=hid[u][:, b:b + 1],
                    scale=1.0,
                )
                ths.append(th)
            # scores = V^T th; M=1 matmuls on distinct column groups run
            # concurrently.  Chunk c -> row 64b+32c.
            for c in range(NCH):
                row = 64 * b + 32 * c
                psSC = pssc.tile([P, CH], f32, tag="psSC", name="pssc")
                for u in range(2):
                    nc.tensor.matmul(
                        psSC[row:row + 1, :],
                        lhsT=vsb[:, u, :],
                        rhs=ths[u][:, CH * c:CH * (c + 1)],
                        start=(u == 0),
                        stop=(u == 1),
                        tile_position=(0, row),
                    )
                nc.vector.tensor_copy(
                    out=sc_sb[b][:, c, j, :, :],
                    in_=psSC.rearrange("p (kk f) -> p kk f", kk=4),
                )

        def scatter_and_exp(g, guard_xbar):
            """Scatter group-g scores (sandwiched against the xbar
            stream), PE-transpose + exp into e128 columns 16g:16g+16."""
            scats = []
            for b in range(BPC):
                for c in range(NCH):
                    row = 64 * b + 32 * c
                    sc = nc.sync.dma_start(
                        out=s32[b][g][8 * c:8 * c + 8, :],
                        in_=sc_sb[b][
                            row:row + 1, c:c + 1, 2 * g:2 * g + 2, :, :
                        ].rearrange("o c jj kk f -> o (c jj kk f)"),
                    )
                    if guard_xbar is not None:
                        tile.add_dep_helper(
                            sc.ins, guard_xbar.ins, sync=True,
                            reason="scatter only after in-flight xbar transposes",
                        )
                    scats.append(sc)
            for b in range(BPC):
                psTS = psmisc.tile([P, 16], f32, tag="misc", name="psts")
                nc.tensor.transpose(
                    out=psTS, in_=s32[b][g], identity=ident[0:16, 0:16]
                )
                nc.scalar.activation(
                    out=e128[b][:, 16 * g:16 * (g + 1)],
                    in_=psTS,
                    func=Act.Exp,
                    scale=1.0,
                )
            return scats

        def ctx_mms(rs):
            # context accumulation; batches on different column groups
            # run concurrently on the PE array.  e128 col r = 16*g + 8*c
            # + 4*jj + k holds exp(score(t = 1024*(2g+jj) + 8*p + 4c+k)).
            for r in rs:
                g, m = divmod(r, 16)
                c, m2 = divmod(m, 8)
                jj, k = divmod(m2, 4)
                j = 2 * g + jj
                fv = 4 * c + k
                for b in range(BPC):
                    nc.tensor.matmul(
                        psC[32 * b:32 * b + 1, D * b:D * (b + 1)],
                        lhsT=e128[b][:, r:r + 1],
                        rhs=vb[b][j][:, :, fv, :],
                        start=(r == 0),
                        stop=(r == 63),
                    )

        # ---- phase 2: per-supertile compute --------------------------
        # j < NPE: PE-transpose path, overlaps the load stream.
        # j >= NPE: xbar path; the transposes run back-to-back on the SP
        # ring after the loads drain.
        ckpt_scats = None
        for j in range(NST):
            for b in range(BPC):
                stream_supertile(b, j)
                if ckpt_scats and j >= NPE:
                    # sandwich: this xbar must wait for the checkpoint
                    # scatters (SBUF->SBUF DMA vs xbar = HW deadlock).
                    for sc in ckpt_scats:
                        tile.add_dep_helper(
                            last_xbar[0].ins, sc.ins, sync=True,
                            reason="xbar transposes resume after scatter batch",
                        )
                    ckpt_scats = None
                jb = (j, b)
                if jb == (1, 1):
                    # groups 0/1 (j 0-3, PE path): no xbar in flight yet.
                    scatter_and_exp(0, None)
                elif jb == (3, 1):
                    scatter_and_exp(1, None)
                elif jb == (4, 0):
                    ctx_mms(range(0, 8))
                elif jb == (4, 1):
                    ctx_mms(range(8, 16))
                elif jb == (5, 0):
                    ctx_mms(range(16, 24))
                elif jb == (5, 1):
                    ctx_mms(range(24, 32))
                elif jb == (7, 0):
                    # group 2 (j 4-5): scores long done; sandwiched
                    # between the (7,0) and (7,1) xbars.
                    ckpt_scats = scatter_and_exp(2, last_xbar[0])
                elif jb == (7, 1):
                    ctx_mms(range(32, 44))

        # ---- tail: group 3 softmax + context + normalize -------------
        ctx_mms(range(44, 48))
        scatter_and_exp(3, last_xbar[0])
        ctx_mms(range(48, 64))

        for b in range(BPC):
            pb = small.tile([P, 1], f32, tag=f"pb_{b}", name=f"pb_{b}")
            nc.vector.reduce_sum(out=pb, in_=e128[b], axis=mybir.AxisListType.X)
            psS = psmisc.tile([P, 1], f32, tag="misc", name="pss")
            nc.tensor.matmul(
                psS[32 * b:32 * b + 1, :], lhsT=ones, rhs=pb, start=True, stop=True
            )
            invS = small.tile([P, 1], f32, tag=f"invs_{b}", name=f"invs_{b}")
            nc.vector.reciprocal(
                out=invS[32 * b:32 * b + 1, :], in_=psS[32 * b:32 * b + 1, :]
            )
            nc.vector.tensor_scalar_mul(
                out=ctx_sb[32 * b:32 * b + 1, :],
                in0=psC[32 * b:32 * b + 1, D * b:D * (b + 1)],
                scalar1=invS[32 * b:32 * b + 1, :],
            )
            nc.sync.dma_start(out=out[b:b + 1, :], in_=ctx_sb[32 * b:32 * b + 1, :])

    nc.finalize()
    return nc


def _run(inputs, trace=False):
    from concourse import bass_utils

    nc = _build()
    in_maps = [
        {
            "query": np.ascontiguousarray(inputs["query"][BPC * i:BPC * (i + 1)]),
            "value": np.ascontiguousarray(inputs["value"][BPC * i:BPC * (i + 1)]),
            "W1": np.asarray(inputs["W1"]),
            "W2": np.asarray(inputs["W2"]),
            "V": np.asarray(inputs["V"]),
        }
        for i in range(NCORES)
    ]
    res = bass_utils.run_bass_kernel_spmd(
        nc, in_maps, core_ids=list(range(NCORES)), trace=trace
    )
    outp = np.concatenate([r["out"] for r in res.results], axis=0)
    return outp.astype(np.float32), res


def kernel(**inputs) -> np.ndarray:
    outp, _ = _run(inputs, trace=False)
    return outp


# revision 26
# speedup vs baseline: 1.1189x; 1.1189x over previous
"""Bahdanau additive attention on 8 Trainium2 NeuronCores.

  score_t = V^T tanh(W1 value_t + W2 query);  out = softmax(score) @ value

Sharding: data-parallel over batch (16 batches -> 2 per core), weights
replicated.

v5: the xbar DMA-transposes are serialized against ALL other DMA traffic
(compiler deadlock guard), and lone loads interleaved between transposes
drop to ~200 GB/s, so the DMA schedule must be phased: all value loads
back-to-back at full rate first, then the xbar transposes.  To shrink
both the idle-PE load phase and the xbar chain:
  * supertiles j<3 (both batches) are transposed ON THE PE ARRAY during
    the load phase (16x 128x128 transpose-mode matmuls + one PSUM->SBUF
    copy per d-half, split DVE/ACT), so their keys/tanh/scores complete
    before the first xbar; only j>=3 (10 supertiles) use the xbar.
  * softmax + context accumulation run incrementally in four j-pair
    groups: score-scatter SBUF->SBUF DMAs execute between xbars on the
    SP ring (dep-sandwiched -- never concurrent with a transpose), so
    the context matmuls overlap the xbar phase instead of serializing
    at the end.
"""

import functools
import os
import sys

import numpy as np

if "/opt/trn_rl_repo" not in sys.path:
    sys.path.insert(0, "/opt/trn_rl_repo")

B, T, D, U = 16, 8192, 256, 256
NCORES = 8
BPC = B // NCORES          # batches per core
P = 128                    # SBUF partitions
ST = 1024                  # t per supertile
NST = T // ST              # supertiles per batch
CH = 512                   # score/keys chunk width (PSUM bank = 512 fp32)
NCH = ST // CH             # chunks per supertile
NPE = 4                    # j < NPE transpose on the PE array (per batch)
NG = 4                     # softmax/context groups (j-pairs)


@functools.lru_cache(maxsize=1)
def _build():
    from contextlib import ExitStack

    import concourse.bass as bass
    import concourse.tile as tile
    from concourse import bacc, mybir
    from concourse.masks import make_identity

    f32 = mybir.dt.float32
    bf16 = mybir.dt.bfloat16
    Act = mybir.ActivationFunctionType

    nc = bacc.Bacc("TRN2", target_bir_lowering=False, debug=False)

    q = nc.dram_tensor("query", [BPC, D], f32, kind="ExternalInput").ap()
    val = nc.dram_tensor("value", [BPC, T, D], f32, kind="ExternalInput").ap()
    w1 = nc.dram_tensor("W1", [D, U], f32, kind="ExternalInput").ap()
    w2 = nc.dram_tensor("W2", [D, U], f32, kind="ExternalInput").ap()
    vv = nc.dram_tensor("V", [U, 1], f32, kind="ExternalInput").ap()
    out = nc.dram_tensor("out", [BPC, D], f32, kind="ExternalOutput").ap()

    with tile.TileContext(nc) as tc, ExitStack() as ctx:
        consts = ctx.enter_context(tc.tile_pool(name="consts", bufs=1))
        vpool = ctx.enter_context(tc.tile_pool(name="vbf", bufs=1))
        v32pool = ctx.enter_context(tc.tile_pool(name="v32", bufs=6))
        vtpool = ctx.enter_context(tc.tile_pool(name="vt", bufs=3))
        thpool = ctx.enter_context(tc.tile_pool(name="th", bufs=3))
        scpool = ctx.enter_context(tc.tile_pool(name="scsb", bufs=1))
        small = ctx.enter_context(tc.tile_pool(name="small", bufs=1))
        psk = ctx.enter_context(tc.tile_pool(name="psk", bufs=2, space="PSUM"))
        pssc = ctx.enter_context(tc.tile_pool(name="pssc", bufs=1, space="PSUM"))
        pst = ctx.enter_context(tc.tile_pool(name="pst", bufs=1, space="PSUM"))
        psctx = ctx.enter_context(tc.tile_pool(name="psctx", bufs=1, space="PSUM"))
        psmisc = ctx.enter_context(tc.tile_pool(name="psmisc", bufs=1, space="PSUM"))

        # ---- constants / weights -------------------------------------
        ident = consts.tile([64, 64], f32)
        make_identity(nc, ident)
        identb = consts.tile([P, P], bf16)
        make_identity(nc, identb)
        ones = consts.tile([P, 1], f32)
        nc.gpsimd.memset(ones, 1.0)
        # preload the exp/tanh ACT table set (~2.7us) during the first
        # value loads instead of stalling the first real tanh on it.
        warm = consts.tile([P, 1], f32)
        nc.scalar.activation(out=warm, in_=warm, func=Act.Tanh, scale=1.0)

        # Small weight loads go FIRST on the SP ring, ahead of the value
        # loads: any DMA issued alongside the saturated value-load stream
        # completes ~25us late and would gate the first PE ops.
        w1tmp = consts.tile([P, 2, U], f32)
        nc.sync.dma_start(out=w1tmp, in_=w1.rearrange("(kb p) u -> p kb u", p=P))
        w1b = consts.tile([P, 2, U], bf16)
        nc.vector.tensor_copy(out=w1b, in_=w1tmp)
        w2b = consts.tile([P, 2, U], f32)
        nc.sync.dma_start(out=w2b, in_=w2.rearrange("(kb p) u -> p kb u", p=P))
        vtmp = consts.tile([P, 2, 1], f32)
        nc.sync.dma_start(out=vtmp, in_=vv.rearrange("(ub p) o -> p ub o", p=P))
        vsb = consts.tile([P, 2, 1], bf16)
        nc.vector.tensor_copy(out=vsb, in_=vtmp)

        # hidden = query @ W2, computed as hidden^T [u, b] so it can feed
        # the tanh as a per-partition bias.
        q_sb = consts.tile([BPC, D], f32)
        nc.sync.dma_start(out=q_sb, in_=q)
        qt = consts.tile([P, 2, BPC], f32)
        for kb in range(2):
            psq = psmisc.tile([P, BPC], f32, tag="misc", name="psq")
            nc.tensor.transpose(
                out=psq, in_=q_sb[:, P * kb:P * (kb + 1)], identity=ident[0:BPC, 0:BPC]
            )
            nc.scalar.copy(out=qt[:, kb, :], in_=psq)
        hid = []
        for u in range(2):
            psh = psmisc.tile([P, BPC], f32, tag="misc", name="psh")
            for kb in range(2):
                nc.tensor.matmul(
                    psh,
                    lhsT=w2b[:, kb, P * u:P * (u + 1)],
                    rhs=qt[:, kb, :],
                    start=(kb == 0),
                    stop=(kb == 1),
                )
            h = consts.tile([P, BPC], f32, tag=f"hid{u}")
            nc.scalar.copy(out=h, in_=psh)
            hid.append(h)

        ctx_sb = consts.tile([P, D], f32)

        # ---- per-core state ------------------------------------------
        # sc_sb[b][p, c, j, kk, 128]: scores, laid out so one scatter
        # source (c, a j-pair, all kk, f) is a single contiguous run.
        # Only rows 64b (chunk c=0) and 64b+32 (c=1) carry data.
        sc_sb = [
            scpool.tile([P, NCH, NST, 4, P], bf16, tag=f"scsb{b}", name=f"scsb{b}")
            for b in range(BPC)
        ]
        # s32[b][g]: scatter target for j-pair group g (j = 2g + jj),
        # row 8*c + 4*jj + k <-> score chunk (t-offset 512c + 128k).
        s32 = [
            [
                small.tile([16, P], bf16, tag=f"s32_{b}_{g}", name=f"s32_{b}_{g}")
                for g in range(NG)
            ]
            for b in range(BPC)
        ]
        e128 = [
            small.tile([P, 64], bf16, tag=f"e_{b}", name=f"e_{b}")
            for b in range(BPC)
        ]
        vb = [[None] * NST for _ in range(BPC)]
        vb32s = {}
        # context accumulator: batch b at [32b : 32b+1, 256b : 256b+256]
        # (one PSUM bank, different col groups so the M=1 chains overlap).
        psC = psctx.tile([P, 2 * D], f32, tag="psC", name="psC")

        last_xbar = [None]

        # ---- phase 1: issue ALL value loads (SP ring, back-to-back) --
        # The xbar transposes serialize against every outstanding DMA, so
        # the loads must form one uninterrupted full-bandwidth stream
        # before the first transpose.
        for j in range(NST):
            for b in range(BPC):
                VB32 = v32pool.tile([P, ST // P, D], f32, tag="vb32", name="vb32")
                vb32s[(b, j)] = VB32
                nc.sync.dma_start(
                    out=VB32,
                    in_=val[b, ST * j:ST * (j + 1), :].rearrange(
                        "(p f) d -> p f d", f=ST // P
                    ),
                )

        def stream_supertile(b, j):
            VB32 = vb32s[(b, j)]
            # bf16 cast; each d-half becomes a contiguous 128-run:
            # VB[p, h, f, d'] = value[ST*j + 8p + f, 128h + d']
            VB = vpool.tile([P, 2, ST // P, P], bf16, tag=f"vb_{b}_{j}")
            vb[b][j] = VB
            nc.vector.tensor_copy(
                out=VB,
                in_=VB32.rearrange("p f (h d) -> p h f d", h=2),
            )
            # VT[d', (h f), t'] = VB[t', h, f, d']
            VT = vtpool.tile([P, 2, ST // P, P], bf16, tag="vt", name="vt")
            if j < NPE:
                # PE transpose-mode path (runs during the load phase,
                # before any xbar): 8 128x128 transposes per d-half into
                # one PSUM bank, then one PSUM->SBUF copy (DVE for h=0,
                # ACT for h=1 so neither engine is the bottleneck).
                for h in range(2):
                    psT = pst.tile([P, ST], bf16, tag="psT", name="psT")
                    for f in range(ST // P):
                        nc.tensor.transpose(
                            out=psT[:, P * f:P * (f + 1)],
                            in_=VB[:, h, f, :],
                            identity=identb,
                        )
                    if h == 0:
                        nc.vector.tensor_copy(out=VT[:, h, :, :], in_=psT)
                    else:
                        nc.scalar.copy(out=VT[:, h, :, :], in_=psT)
            else:
                tr = nc.sync.dma_start(
                    out=VT.rearrange("p h f t -> p (h f) t"),
                    in_=VB.rearrange("p h f t -> p (h f t)"),
                    transpose=True,
                )
                last_xbar[0] = tr
            # keys^T = W1^T @ value^T, one u-half at a time; tanh with the
            # query bias fused reads each 2-bank psK tile in one op.
            ths = []
            for u in range(2):
                psK = psk.tile([P, ST], f32, tag="psK", name="psk")
                for c in range(NCH):
                    for kb in range(2):
                        nc.tensor.matmul(
                            psK[:, CH * c:CH * (c + 1)],
                            lhsT=w1b[:, kb, P * u:P * (u + 1)],
                            rhs=VT[
                                :, kb, (CH // P) * c:(CH // P) * (c + 1), :
                            ].rearrange("p f t -> p (f t)"),
                            start=(kb == 0),
                            stop=(kb == 1),
                        )
                th = thpool.tile([P, ST], bf16, tag="th", name="th")
                nc.scalar.activation(
                    out=th,
                    in_=psK,
                    func=Act.Tanh,
                    bias=hid[u][:, b:b + 1],
                    scale=1.0,
                )
                ths.append(th)
            # scores = V^T th; M=1 matmuls on distinct column groups run
            # concurrently.  Chunk c -> row 64b+32c.
            for c in range(NCH):
                row = 64 * b + 32 * c
                psSC = pssc.tile([P, CH], f32, tag="psSC", name="pssc")
                for u in range(2):
                    nc.tensor.matmul(
                        psSC[row:row + 1, :],
                        lhsT=vsb[:, u, :],
                        rhs=ths[u][:, CH * c:CH * (c + 1)],
                        start=(u == 0),
                        stop=(u == 1),
                        tile_position=(0, row),
                    )
                nc.vector.tensor_copy(
                    out=sc_sb[b][:, c, j, :, :],
                    in_=psSC.rearrange("p (kk f) -> p kk f", kk=4),
                )

        def scatter_and_exp(g, guard_xbar):
            """Scatter group-g scores (sandwiched against the xbar
            stream), PE-transpose + exp into e128 columns 16g:16g+16."""
            scats = []
            for b in range(BPC):
                for c in range(NCH):
                    row = 64 * b + 32 * c
                    sc = nc.sync.dma_start(
                        out=s32[b][g][8 * c:8 * c + 8, :],
                        in_=sc_sb[b][
                            row:row + 1, c:c + 1, 2 * g:2 * g + 2, :, :
                        ].rearrange("o c jj kk f -> o (c jj kk f)"),
                    )
                    if guard_xbar is not None:
                        tile.add_dep_helper(
                            sc.ins, guard_xbar.ins, sync=True,
                            reason="scatter only after in-flight xbar transposes",
                        )
                    scats.append(sc)
            for b in range(BPC):
                # bf16 transpose output shares the (bank-aligned) psT slot
                psTS = pst.tile([P, ST], bf16, tag="psT", name="psts")
                nc.tensor.transpose(
                    out=psTS[:, 0:16], in_=s32[b][g], identity=identb[0:16, 0:16]
                )
                nc.scalar.activation(
                    out=e128[b][:, 16 * g:16 * (g + 1)],
                    in_=psTS[:, 0:16],
                    func=Act.Exp,
                    scale=1.0,
                )
            return scats

        def ctx_mms(rs):
            # context accumulation; batches on different column groups
            # run concurrently on the PE array.  e128 col r = 16*g + 8*c
            # + 4*jj + k holds exp(score(t = 1024*(2g+jj) + 8*p + 4c+k)).
            for r in rs:
                g, m = divmod(r, 16)
                c, m2 = divmod(m, 8)
                jj, k = divmod(m2, 4)
                j = 2 * g + jj
                fv = 4 * c + k
                for b in range(BPC):
                    nc.tensor.matmul(
                        psC[32 * b:32 * b + 1, D * b:D * (b + 1)],
                        lhsT=e128[b][:, r:r + 1],
                        rhs=vb[b][j][:, :, fv, :],
                        start=(r == 0),
                        stop=(r == 63),
                    )

        # ---- phase 2: per-supertile compute --------------------------
        # j < NPE: PE-transpose path, overlaps the load stream.
        # j >= NPE: xbar path; the transposes run back-to-back on the SP
        # ring after the loads drain.
        ckpt_scats = None
        for j in range(NST):
            for b in range(BPC):
                stream_supertile(b, j)
                if ckpt_scats and j >= NPE:
                    # sandwich: this xbar must wait for the checkpoint
                    # scatters (SBUF->SBUF DMA vs xbar = HW deadlock).
                    for sc in ckpt_scats:
                        tile.add_dep_helper(
                            last_xbar[0].ins, sc.ins, sync=True,
                            reason="xbar transposes resume after scatter batch",
                        )
                    ckpt_scats = None
                jb = (j, b)
                if jb == (1, 1):
                    # groups 0/1 (j 0-3, PE path): no xbar in flight yet.
                    scatter_and_exp(0, None)
                elif jb == (3, 1):
                    scatter_and_exp(1, None)
                elif jb == (4, 0):
                    ctx_mms(range(0, 8))
                elif jb == (4, 1):
                    ctx_mms(range(8, 16))
                elif jb == (5, 0):
                    ctx_mms(range(16, 24))
                elif jb == (5, 1):
                    ctx_mms(range(24, 32))
                elif jb == (7, 0):
                    # group 2 (j 4-5): scores long done; sandwiched
                    # between the (7,0) and (7,1) xbars.
                    ckpt_scats = scatter_and_exp(2, last_xbar[0])
                elif jb == (7, 1):
                    ctx_mms(range(32, 44))

        # ---- tail: group 3 softmax + context + normalize -------------
        ctx_mms(range(44, 48))
        scatter_and_exp(3, last_xbar[0])
        ctx_mms(range(48, 64))

        for b in range(BPC):
            pb = small.tile([P, 1], f32, tag=f"pb_{b}", name=f"pb_{b}")
            nc.vector.reduce_sum(out=pb, in_=e128[b], axis=mybir.AxisListType.X)
            psS = psmisc.tile([P, 1], f32, tag="misc", name="pss")
            nc.tensor.matmul(
                psS[32 * b:32 * b + 1, :], lhsT=ones, rhs=pb, start=True, stop=True
            )
            invS = small.tile([P, 1], f32, tag=f"invs_{b}", name=f"invs_{b}")
            nc.vector.reciprocal(
                out=invS[32 * b:32 * b + 1, :], in_=psS[32 * b:32 * b + 1, :]
            )
            nc.vector.tensor_scalar_mul(
                out=ctx_sb[32 * b:32 * b + 1, :],
                in0=psC[32 * b:32 * b + 1, D * b:D * (b + 1)],
                scalar1=invS[32 * b:32 * b + 1, :],
            )
            nc.sync.dma_start(out=out[b:b + 1, :], in_=ctx_sb[32 * b:32 * b + 1, :])

    nc.finalize()
    return nc


def _run(inputs, trace=False):
    from concourse import bass_utils

    nc = _build()
    in_maps = [
        {
            "query": np.ascontiguousarray(inputs["query"][BPC * i:BPC * (i + 1)]),
            "value": np.ascontiguousarray(inputs["value"][BPC * i:BPC * (i + 1)]),
            "W1": np.asarray(inputs["W1"]),
            "W2": np.asarray(inputs["W2"]),
            "V": np.asarray(inputs["V"]),
        }
        for i in range(NCORES)
    ]
    res = bass_utils.run_bass_kernel_spmd(
        nc, in_maps, core_ids=list(range(NCORES)), trace=trace
    )
    outp = np.concatenate([r["out"] for r in res.results], axis=0)
    return outp.astype(np.float32), res


def kernel(**inputs) -> np.ndarray:
    outp, _ = _run(inputs, trace=False)
    return outp
